# revision 1
# baseline (speedup 1.0000x reference)
"""Conv1dLoRA Trainium2 kernel.

Computes, per sample s:
  A[s] = MLP_A(a_emb[s]) in [64, 8]   (Linear-BN-GELU-Linear)
  B[s] = MLP_B(b_emb[s]) in [8, 192]
  W[s] = A[s] @ B[s]  -> per-sample conv weight [cin=64, cout*K=192]
  Y[s] = conv1d(X[s], W[s]*SCALE + base_w, pad=1) + base_b

Sharding: data-parallel over batch. 128 samples -> 16 per core x 8 cores.
MLP/base params are replicated; small host-side numpy transposes put every
weight into the exact SBUF layout the PE needs (no on-device transposes).

Device program (identical SPMD program on all 8 cores):
  - MLPs batched over the 16 local samples; BN+GELU fused into one ACT op.
  - W[s] via 8 rank-1 (K=1) PE matmuls per sample, two samples packed
    block-diagonally into one [128, 384] PSUM tile; base_w folded in with a
    DVE add (so base conv + lora conv become ONE conv).
  - Conv: per 512-col chunk, 3 shifted matmuls (taps) accumulate in PSUM,
    float32r mode (full-rate fp32 on the PE at N=512), 2 samples per matmul
    via the block-diagonal weights. Zero-padded halo columns in SBUF.
  - Bias add fused into the PSUM->SBUF copy (DVE tensor_scalar).
"""

import numpy as np

BS, CIN, COUT, L = 128, 64, 64, 8192
K, R, GROUPS = 3, 8, 1
EMB, HID = 256, 256
BN_EPS = 1e-5
NCORES = 8
SH = BS // NCORES          # 16 samples per core
NPAIR = SH // 2            # 8 sample pairs per core
LCH = 512                  # conv chunk (one PSUM bank of fp32)
NCH = L // LCH             # 16 chunks
KCO = K * COUT             # 192 = per-sample W columns (k-major: k*64+cout)

_NC = None                 # cached compiled Bass program


def _build_program():
    import concourse.tile as tile
    from concourse import bacc, mybir

    f32 = mybir.dt.float32
    f32r = mybir.dt.float32r
    bf16 = mybir.dt.bfloat16
    AF = mybir.ActivationFunctionType

    nc = bacc.Bacc(
        "TRN2",
        target_bir_lowering=False,
        debug=False,
        enable_asserts=False,
        num_devices=NCORES,
    )

    dt_in = lambda name, shape: nc.dram_tensor(name, shape, f32, kind="ExternalInput").ap()

    X = dt_in("X", [SH, CIN, L])
    aT = dt_in("aT", [EMB, SH])
    bT = dt_in("bT", [EMB, SH])
    Aw1T = dt_in("Aw1T", [EMB, HID])       # [e, h]
    Bw1T = dt_in("Bw1T", [EMB, HID])
    Aw2T = dt_in("Aw2T", [HID, CIN * R])   # [h, m'] m' = r*64+i (r-major)
    Bw2T = dt_in("Bw2T", [HID, R * KCO])   # [h, m'] m' = r*192 + k*64 + cout
    vecs = dt_in("vecs", [128, 9])         # gA0 gA1 cA0 cA1 gB0 gB1 cB0 cB1 bias
    b2A = dt_in("b2A", [1, CIN * R])       # layer-2 bias rows (permuted)
    b2B = dt_in("b2B", [1, R * KCO])
    base_pair = dt_in("base_pair", [128, 2 * KCO])  # tap-major block-diag base_w
    Y = nc.dram_tensor("Y", [SH, COUT, L], f32, kind="ExternalOutput").ap()

    with tile.TileContext(nc) as tc:
        with (
            tc.tile_pool(name="const", bufs=1) as const,
            # conv-phase pools opened first so their SBUF/PSUM addresses are
            # never reused from transient pools (address reuse would add
            # write-after-read deps that stall the conv stream)
            tc.tile_pool(name="yps", bufs=4, space="PSUM") as yps,
            tc.tile_pool(name="xpool", bufs=4) as xpool,
            tc.tile_pool(name="ypool", bufs=3) as ypool,
            tc.tile_pool(name="wpool", bufs=NPAIR) as wpool,
            tc.tile_pool(name="wps", bufs=2, space="PSUM") as wps,
            tc.tile_pool(name="stage", bufs=2) as stage,
        ):
            # ---- constants; MLP-critical ones first, one DMA for all vectors ----
            def load(name, src_ap, shape, eng=None):
                t = const.tile(list(shape), f32, name=name, tag=name)
                (eng or nc.sync).dma_start(t[:], src_ap)
                return t

            def load16(name, src_ap, shape):
                t = const.tile(list(shape), bf16, name=name, tag=name)
                nc.gpsimd.dma_start(t[:], src_ap)  # SWDGE casts f32 -> bf16
                return t

            aT_sb = [load(f"aT{e}", aT[e * 128:(e + 1) * 128], (128, SH)) for e in range(2)]
            bT_sb = [load(f"bT{e}", bT[e * 128:(e + 1) * 128], (128, SH)) for e in range(2)]
            vecs_sb = load("vecs", vecs, (128, 9))
            gA_sb = [vecs_sb[:, h:h + 1] for h in range(2)]
            cA_sb = [vecs_sb[:, 2 + h:3 + h] for h in range(2)]
            gB_sb = [vecs_sb[:, 4 + h:5 + h] for h in range(2)]
            cB_sb = [vecs_sb[:, 6 + h:7 + h] for h in range(2)]
            bias_sb = vecs_sb[:, 8:9]
            Aw1T_sb = [load(f"Aw1T{e}", Aw1T[e * 128:(e + 1) * 128], (128, HID)) for e in range(2)]
            Bw1T_sb = [load(f"Bw1T{e}", Bw1T[e * 128:(e + 1) * 128], (128, HID)) for e in range(2)]
            Aw2T_sb = [load16(f"Aw2T{h}", Aw2T[h * 128:(h + 1) * 128], (128, CIN * R)) for h in range(2)]
            Bw2T_sb = [load16(f"Bw2T{h}", Bw2T[h * 128:(h + 1) * 128], (128, R * KCO)) for h in range(2)]
            b2A_sb = load16("b2A", b2A, (1, CIN * R))
            b2B_sb = load16("b2B", b2B, (1, R * KCO))
            base_sb = load("base_pair", base_pair, (128, 2 * KCO), eng=nc.scalar)
            ones_sb = const.tile([1, SH], bf16, name="ones", tag="ones")
            nc.vector.memset(ones_sb[:], 1.0)

            A_row = const.tile([SH, CIN * R], bf16, name="A_row", tag="A_row")
            B_row = const.tile([SH, R * KCO], bf16, name="B_row", tag="B_row")

            # ---- MLPs (batched over the 16 local samples) ----
            with tc.tile_pool(name="mps", bufs=2, space="PSUM") as mps:
                gel = {}
                for side, w1T, embT, g_sb, c_sb in (
                    ("A", Aw1T_sb, aT_sb, gA_sb, cA_sb),
                    ("B", Bw1T_sb, bT_sb, gB_sb, cB_sb),
                ):
                    for hc in range(2):
                        ps1 = mps.tile([128, SH], f32, name=f"ps1{side}{hc}", tag="mlp")
                        for ec in range(2):
                            nc.tensor.matmul(
                                ps1[:],
                                w1T[ec][:, hc * 128:(hc + 1) * 128],
                                embT[ec][:],
                                start=(ec == 0),
                                stop=(ec == 1),
                            )
                        g = const.tile([128, SH], bf16, name=f"gel{side}{hc}", tag=f"gel{side}{hc}")
                        # gelu(h * g' + (b1*g' + beta)) == BN+bias+GELU fused
                        nc.scalar.activation(
                            g[:], ps1[:], AF.Gelu, bias=c_sb[hc][:], scale=g_sb[hc][:]
                        )
                        gel[(side, hc)] = g

                for side, w2T_sb, b2_sb, dst, width in (
                    ("A", Aw2T_sb, b2A_sb, A_row, CIN * R),
                    ("B", Bw2T_sb, b2B_sb, B_row, R * KCO),
                ):
                    for nb in range(width // 512):
                        ps2 = mps.tile([SH, 512], f32, name=f"ps2{side}{nb}", tag="mlp")
                        for hc in range(2):
                            nc.tensor.matmul(
                                ps2[:],
                                gel[(side, hc)][:],
                                w2T_sb[hc][:, nb * 512:(nb + 1) * 512],
                                start=(hc == 0),
                                stop=False,
                            )
                        # + layer-2 bias via rank-1 ones matmul
                        nc.tensor.matmul(
                            ps2[:],
                            ones_sb[:],
                            b2_sb[:, nb * 512:(nb + 1) * 512],
                            start=False,
                            stop=True,
                        )
                        nc.vector.tensor_copy(dst[:, nb * 512:(nb + 1) * 512], ps2[:])

            # ---- conv stream; W generation software-pipelined one pair
            # ---- ahead so pair boundaries never stall the PE
            OB = 2048                      # output block columns (1MB DMAs)
            XB = 2048                      # X load chunk columns
            wpks = {}

            def emit_w(t):
                # stage this pair's A/B rows at partitions 0 (j=0) and 64
                # (j=1) — compute engines need 32-aligned partition bases
                ast = stage.tile([128, CIN * R], bf16, name=f"ast{t}", tag="ast")
                bst = stage.tile([128, R * KCO], bf16, name=f"bst{t}", tag="bst")
                ast2 = ast[:].rearrange("(a b) f -> a b f", a=2)[:, 0, :]
                bst2 = bst[:].rearrange("(a b) f -> a b f", a=2)[:, 0, :]
                nc.scalar.dma_start(ast2, A_row[2 * t:2 * t + 2, :])
                nc.scalar.dma_start(bst2, B_row[2 * t:2 * t + 2, :])

                # W[s] = A[s] @ B[s], two samples block-diagonal
                psw = wps.tile([128, 2 * KCO], f32, name=f"psw{t}", tag="psw")
                nc.vector.memset(psw[:], 0.0)
                for j in range(2):
                    for r in range(R):
                        nc.tensor.matmul(
                            psw[j * 64:(j + 1) * 64, j * KCO:(j + 1) * KCO],
                            ast[j * 64:j * 64 + 1, r * 64:(r + 1) * 64],
                            bst[j * 64:j * 64 + 1, r * KCO:(r + 1) * KCO],
                            start=(r == 0),
                            stop=(r == R - 1),
                        )
                # repack (j, k, c) -> tap-major (k, j, c) columns adding
                # base_w, so each tap's lhsT is one contiguous [128, 128]
                wpk = wpool.tile([128, 2 * KCO], bf16, name=f"wpk{t}", tag="wpk")
                for j in range(2):
                    for k in range(K):
                        dst = k * 128 + j * 64
                        srcc = j * KCO + k * 64
                        nc.vector.tensor_add(
                            wpk[:, dst:dst + 64],
                            psw[:, srcc:srcc + 64],
                            base_sb[:, dst:dst + 64],
                        )
                wpks[t] = wpk

            for t in range(NPAIR):
                emit_w(t)
                wpk = wpks[t]
                xp = xpool.tile([128, L + 2], bf16, name=f"xp{t}", tag="xp")
                nc.vector.memset(xp[:, 0:1], 0.0)
                nc.vector.memset(xp[:, L + 1:L + 2], 0.0)
                # XB-column chunks on the gpsimd casting path (f32->bf16)
                for xb in range(L // XB):
                    nc.gpsimd.dma_start(
                        xp[:, 1 + xb * XB:1 + (xb + 1) * XB],
                        X[2 * t:2 * t + 2, :, xb * XB:(xb + 1) * XB],
                    )
                for ob in range(L // OB):
                    yo = ypool.tile([128, OB], f32, name=f"yo{t}_{ob}", tag="yo")
                    for cc in range(OB // LCH):
                        c = ob * (OB // LCH) + cc
                        yp = yps.tile([128, LCH], f32, name=f"yp{t}_{c}", tag="yp")
                        for k in range(K):
                            nc.tensor.matmul(
                                yp[:],
                                wpk[:, k * 128:(k + 1) * 128],
                                xp[:, c * LCH + k:c * LCH + k + LCH],
                                start=(k == 0),
                                stop=(k == K - 1),
                            )
                        # bias fused into the PSUM->SBUF copy, alternating
                        # DVE / ACT so neither engine is the bottleneck
                        if c % 2 == 0:
                            nc.vector.tensor_scalar_add(
                                yo[:, cc * LCH:(cc + 1) * LCH], yp[:], bias_sb[:]
                            )
                        else:
                            nc.scalar.activation(
                                yo[:, cc * LCH:(cc + 1) * LCH], yp[:],
                                AF.Identity, bias=bias_sb[:],
                            )
                    # ~1MB output blocks, alternating HWDGE rings
                    eng = nc.sync if ob % 2 == 0 else nc.scalar
                    lo, hi = ob * OB, (ob + 1) * OB
                    eng.dma_start(Y[2 * t, :, lo:hi], yo[0:64, :])
                    eng.dma_start(Y[2 * t + 1, :, lo:hi], yo[64:128, :])

    nc.compile()
    return nc


def _host_prep(inputs):
    """Shared (replicated) tensors, in device layouts. Returns dict of np arrays."""
    f = np.float32
    gA_flat = (inputs["A_bn_g"] / np.sqrt(f(1.0) + f(BN_EPS))).astype(f)
    gB_flat = (inputs["B_bn_g"] / np.sqrt(f(1.0) + f(BN_EPS))).astype(f)
    cA_flat = (inputs["A_b1"] * gA_flat + inputs["A_bn_b"]).astype(f)
    cB_flat = (inputs["B_b1"] * gB_flat + inputs["B_bn_b"]).astype(f)

    # A layer-2: columns m = i*8+r  ->  m' = r*64+i (r-major)
    permA = (np.arange(R)[:, None] + np.arange(CIN)[None, :] * R).reshape(-1)  # m'[r,i] -> i*8+r
    Aw2T = np.ascontiguousarray(inputs["A_w2"].T[:, permA], dtype=f)
    b2A = np.ascontiguousarray(inputs["A_b2"][permA], dtype=f).reshape(1, CIN * R)

    # B layer-2: columns m = r*192 + cout*3 + k  ->  m' = r*192 + k*64 + cout
    m2 = (np.arange(COUT)[None, :] * K + np.arange(K)[:, None]).reshape(-1)  # m2'[k,c] -> c*3+k
    permB = (np.arange(R)[:, None] * KCO + m2[None, :]).reshape(-1)
    Bw2T = np.ascontiguousarray(inputs["B_w2"].T[:, permB], dtype=f)
    b2B = np.ascontiguousarray(inputs["B_b2"][permB], dtype=f).reshape(1, R * KCO)

    # base_w [cout, cin, k] -> tap-major block-diag pair layout:
    # base_pair[j*64 + i, k*128 + j*64 + c] = base_w[c, i, k]
    base_pair = np.zeros((128, 2 * KCO), dtype=f)
    for j in range(2):
        for k in range(K):
            base_pair[j * 64:(j + 1) * 64, k * 128 + j * 64:k * 128 + j * 64 + 64] = (
                inputs["base_w"][:, :, k].T.astype(f)
            )

    bias_out = np.concatenate([inputs["base_b"], inputs["base_b"]]).astype(f)

    # all per-partition vectors in one tensor -> one early DMA:
    # cols = gA0 gA1 cA0 cA1 gB0 gB1 cB0 cB1 bias_out
    vecs = np.stack([
        gA_flat[:128], gA_flat[128:], cA_flat[:128], cA_flat[128:],
        gB_flat[:128], gB_flat[128:], cB_flat[:128], cB_flat[128:],
        bias_out,
    ], axis=1).astype(f)

    return {
        "Aw1T": np.ascontiguousarray(inputs["A_w1"].T, dtype=f),
        "Bw1T": np.ascontiguousarray(inputs["B_w1"].T, dtype=f),
        "Aw2T": Aw2T,
        "Bw2T": Bw2T,
        "vecs": vecs,
        "b2A": b2A,
        "b2B": b2B,
        "base_pair": base_pair,
    }


def _in_maps(inputs):
    shared = _host_prep(inputs)
    f = np.float32
    maps = []
    for c in range(NCORES):
        lo, hi = c * SH, (c + 1) * SH
        m = dict(shared)
        m["X"] = np.ascontiguousarray(inputs["X"][lo:hi], dtype=f)
        m["aT"] = np.ascontiguousarray(inputs["a_embedding"][lo:hi].T, dtype=f)
        m["bT"] = np.ascontiguousarray(inputs["b_embedding"][lo:hi].T, dtype=f)
        maps.append(m)
    return maps


def run(inputs, trace=False):
    """Run the kernel; returns (Y_full, BassKernelResults)."""
    global _NC
    if _NC is None:
        _NC = _build_program()
    from concourse.bass_utils import run_bass_kernel_spmd

    res = run_bass_kernel_spmd(
        _NC, _in_maps(inputs), core_ids=list(range(NCORES)), trace=trace
    )
    Y = np.concatenate([r["Y"] for r in res.results], axis=0)
    return Y, res


def kernel(**inputs) -> np.ndarray:
    Y, _ = run(inputs, trace=False)
    return Y



# revision 8
# speedup vs baseline: 1.2357x; 1.2357x over previous
"""Conv1dLoRA Trainium2 kernel.

Computes, per sample s:
  A[s] = MLP_A(a_emb[s]) in [64, 8]   (Linear-BN-GELU-Linear)
  B[s] = MLP_B(b_emb[s]) in [8, 192]
  W[s] = A[s] @ B[s]  -> per-sample conv weight [cin=64, cout*K=192]
  Y[s] = conv1d(X[s], W[s]*SCALE + base_w, pad=1) + base_b

Sharding: data-parallel over batch. 128 samples -> 16 per core x 8 cores.
MLP/base params are replicated; small host-side numpy transposes put every
weight into the exact SBUF layout the PE needs (no on-device transposes).

Device program (identical SPMD program on all 8 cores):
  - MLPs batched over the 16 local samples; BN+GELU fused into one ACT op.
  - W[s] via 8 rank-1 (K=1) PE matmuls per sample, two samples packed
    block-diagonally into one [128, 384] PSUM tile; base_w folded in with a
    DVE add (so base conv + lora conv become ONE conv).
  - Conv: per 512-col chunk, 3 shifted matmuls (taps) accumulate in PSUM,
    float32r mode (full-rate fp32 on the PE at N=512), 2 samples per matmul
    via the block-diagonal weights. Zero-padded halo columns in SBUF.
  - Bias add fused into the PSUM->SBUF copy (DVE tensor_scalar).
"""

import numpy as np
import ml_dtypes

BF16 = ml_dtypes.bfloat16

BS, CIN, COUT, L = 128, 64, 64, 8192
K, R, GROUPS = 3, 8, 1
EMB, HID = 256, 256
BN_EPS = 1e-5
NCORES = 8
SH = BS // NCORES          # 16 samples per core
NPAIR = SH // 2            # 8 sample pairs per core
LCH = 512                  # conv chunk (one PSUM bank of fp32)
NCH = L // LCH             # 16 chunks
KCO = K * COUT             # 192 = per-sample W columns (k-major: k*64+cout)

_NC = None                 # cached compiled Bass program


def _build_program():
    import concourse.tile as tile
    from concourse import bacc, mybir

    f32 = mybir.dt.float32
    f32r = mybir.dt.float32r
    bf16 = mybir.dt.bfloat16
    AF = mybir.ActivationFunctionType

    nc = bacc.Bacc(
        "TRN2",
        target_bir_lowering=False,
        debug=False,
        enable_asserts=False,
        num_devices=NCORES,
    )

    dt_in = lambda name, shape: nc.dram_tensor(name, shape, f32, kind="ExternalInput").ap()

    # X arrives pre-cast to bf16 on the host: halves the HBM read traffic.
    X = nc.dram_tensor("X", [SH, CIN, L], bf16, kind="ExternalInput").ap()
    aT = dt_in("aT", [EMB, SH])
    bT = dt_in("bT", [EMB, SH])
    Aw1T = dt_in("Aw1T", [EMB, HID])       # [e, h]
    Bw1T = dt_in("Bw1T", [EMB, HID])
    Aw2T = dt_in("Aw2T", [HID, CIN * R])   # [h, m'] m' = r*64+i (r-major)
    Bw2T = dt_in("Bw2T", [HID, R * KCO])   # [h, m'] m' = r*192 + k*64 + cout
    vecs = dt_in("vecs", [128, 9])         # gA0 gA1 cA0 cA1 gB0 gB1 cB0 cB1 bias
    b2A = dt_in("b2A", [1, CIN * R])       # layer-2 bias rows (permuted)
    b2B = dt_in("b2B", [1, R * KCO])
    base_pair = dt_in("base_pair", [128, 2 * KCO])  # tap-major block-diag base_w
    # Y leaves as bf16 (upcast to f32 on the host): halves the HBM write traffic.
    Y = nc.dram_tensor("Y", [SH, COUT, L], bf16, kind="ExternalOutput").ap()

    with tile.TileContext(nc) as tc:
        with (
            tc.tile_pool(name="const", bufs=1) as const,
            # conv-phase pools opened first so their SBUF/PSUM addresses are
            # never reused from transient pools (address reuse would add
            # write-after-read deps that stall the conv stream)
            tc.tile_pool(name="yps", bufs=4, space="PSUM") as yps,
            tc.tile_pool(name="xpool", bufs=4) as xpool,
            tc.tile_pool(name="ypool", bufs=3) as ypool,
            tc.tile_pool(name="wpool", bufs=NPAIR) as wpool,
            tc.tile_pool(name="wps", bufs=2, space="PSUM") as wps,
            tc.tile_pool(name="stage", bufs=2) as stage,
        ):
            # ---- constants; MLP-critical ones first, one DMA for all vectors ----
            def load(name, src_ap, shape, eng=None):
                t = const.tile(list(shape), f32, name=name, tag=name)
                (eng or nc.sync).dma_start(t[:], src_ap)
                return t

            def load16(name, src_ap, shape):
                t = const.tile(list(shape), bf16, name=name, tag=name)
                nc.gpsimd.dma_start(t[:], src_ap)  # SWDGE casts f32 -> bf16
                return t

            aT_sb = [load(f"aT{e}", aT[e * 128:(e + 1) * 128], (128, SH)) for e in range(2)]
            bT_sb = [load(f"bT{e}", bT[e * 128:(e + 1) * 128], (128, SH)) for e in range(2)]
            vecs_sb = load("vecs", vecs, (128, 9))
            gA_sb = [vecs_sb[:, h:h + 1] for h in range(2)]
            cA_sb = [vecs_sb[:, 2 + h:3 + h] for h in range(2)]
            gB_sb = [vecs_sb[:, 4 + h:5 + h] for h in range(2)]
            cB_sb = [vecs_sb[:, 6 + h:7 + h] for h in range(2)]
            bias_sb = vecs_sb[:, 8:9]
            Aw1T_sb = [load(f"Aw1T{e}", Aw1T[e * 128:(e + 1) * 128], (128, HID)) for e in range(2)]
            Bw1T_sb = [load(f"Bw1T{e}", Bw1T[e * 128:(e + 1) * 128], (128, HID)) for e in range(2)]
            Aw2T_sb = [load16(f"Aw2T{h}", Aw2T[h * 128:(h + 1) * 128], (128, CIN * R)) for h in range(2)]
            Bw2T_sb = [load16(f"Bw2T{h}", Bw2T[h * 128:(h + 1) * 128], (128, R * KCO)) for h in range(2)]
            b2A_sb = load16("b2A", b2A, (1, CIN * R))
            b2B_sb = load16("b2B", b2B, (1, R * KCO))
            base_sb = load("base_pair", base_pair, (128, 2 * KCO), eng=nc.scalar)
            ones_sb = const.tile([1, SH], bf16, name="ones", tag="ones")
            nc.vector.memset(ones_sb[:], 1.0)

            A_row = const.tile([SH, CIN * R], bf16, name="A_row", tag="A_row")
            B_row = const.tile([SH, R * KCO], bf16, name="B_row", tag="B_row")

            # ---- MLPs (batched over the 16 local samples) ----
            with tc.tile_pool(name="mps", bufs=2, space="PSUM") as mps:
                gel = {}
                for side, w1T, embT, g_sb, c_sb in (
                    ("A", Aw1T_sb, aT_sb, gA_sb, cA_sb),
                    ("B", Bw1T_sb, bT_sb, gB_sb, cB_sb),
                ):
                    for hc in range(2):
                        ps1 = mps.tile([128, SH], f32, name=f"ps1{side}{hc}", tag="mlp")
                        for ec in range(2):
                            nc.tensor.matmul(
                                ps1[:],
                                w1T[ec][:, hc * 128:(hc + 1) * 128],
                                embT[ec][:],
                                start=(ec == 0),
                                stop=(ec == 1),
                            )
                        g = const.tile([128, SH], bf16, name=f"gel{side}{hc}", tag=f"gel{side}{hc}")
                        # gelu(h * g' + (b1*g' + beta)) == BN+bias+GELU fused
                        nc.scalar.activation(
                            g[:], ps1[:], AF.Gelu, bias=c_sb[hc][:], scale=g_sb[hc][:]
                        )
                        gel[(side, hc)] = g

                for side, w2T_sb, b2_sb, dst, width in (
                    ("A", Aw2T_sb, b2A_sb, A_row, CIN * R),
                    ("B", Bw2T_sb, b2B_sb, B_row, R * KCO),
                ):
                    for nb in range(width // 512):
                        ps2 = mps.tile([SH, 512], f32, name=f"ps2{side}{nb}", tag="mlp")
                        for hc in range(2):
                            nc.tensor.matmul(
                                ps2[:],
                                gel[(side, hc)][:],
                                w2T_sb[hc][:, nb * 512:(nb + 1) * 512],
                                start=(hc == 0),
                                stop=False,
                            )
                        # + layer-2 bias via rank-1 ones matmul
                        nc.tensor.matmul(
                            ps2[:],
                            ones_sb[:],
                            b2_sb[:, nb * 512:(nb + 1) * 512],
                            start=False,
                            stop=True,
                        )
                        nc.vector.tensor_copy(dst[:, nb * 512:(nb + 1) * 512], ps2[:])

            # ---- conv stream; W generation software-pipelined one pair
            # ---- ahead so pair boundaries never stall the PE
            OB = 2048                      # output block columns (0.5MB bf16 DMAs)
            XB = 4096                      # X load chunk columns (1MB bf16 DMAs)
            wpks = {}

            def emit_w(t):
                # stage this pair's A/B rows at partitions 0 (j=0) and 64
                # (j=1) — compute engines need 32-aligned partition bases
                ast = stage.tile([128, CIN * R], bf16, name=f"ast{t}", tag="ast")
                bst = stage.tile([128, R * KCO], bf16, name=f"bst{t}", tag="bst")
                ast2 = ast[:].rearrange("(a b) f -> a b f", a=2)[:, 0, :]
                bst2 = bst[:].rearrange("(a b) f -> a b f", a=2)[:, 0, :]
                nc.scalar.dma_start(ast2, A_row[2 * t:2 * t + 2, :])
                nc.scalar.dma_start(bst2, B_row[2 * t:2 * t + 2, :])

                # W[s] = A[s] @ B[s], two samples block-diagonal
                psw = wps.tile([128, 2 * KCO], f32, name=f"psw{t}", tag="psw")
                nc.vector.memset(psw[:], 0.0)
                for j in range(2):
                    for r in range(R):
                        nc.tensor.matmul(
                            psw[j * 64:(j + 1) * 64, j * KCO:(j + 1) * KCO],
                            ast[j * 64:j * 64 + 1, r * 64:(r + 1) * 64],
                            bst[j * 64:j * 64 + 1, r * KCO:(r + 1) * KCO],
                            start=(r == 0),
                            stop=(r == R - 1),
                        )
                # repack (j, k, c) -> tap-major (k, j, c) columns adding
                # base_w, so each tap's lhsT is one contiguous [128, 128]
                wpk = wpool.tile([128, 2 * KCO], bf16, name=f"wpk{t}", tag="wpk")
                for j in range(2):
                    for k in range(K):
                        dst = k * 128 + j * 64
                        srcc = j * KCO + k * 64
                        nc.vector.tensor_add(
                            wpk[:, dst:dst + 64],
                            psw[:, srcc:srcc + 64],
                            base_sb[:, dst:dst + 64],
                        )
                wpks[t] = wpk

            for t in range(NPAIR):
                emit_w(t)
                wpk = wpks[t]
                xp = xpool.tile([128, L + 2], bf16, name=f"xp{t}", tag="xp")
                nc.vector.memset(xp[:, 0:1], 0.0)
                nc.vector.memset(xp[:, L + 1:L + 2], 0.0)
                # 1MB X chunks on the sync HWDGE ring (input is already bf16)
                for xb in range(L // XB):
                    nc.sync.dma_start(
                        xp[:, 1 + xb * XB:1 + (xb + 1) * XB],
                        X[2 * t:2 * t + 2, :, xb * XB:(xb + 1) * XB],
                    )
                for ob in range(L // OB):
                    yo = ypool.tile([128, OB], bf16, name=f"yo{t}_{ob}", tag="yo")
                    for cc in range(OB // LCH):
                        c = ob * (OB // LCH) + cc
                        yp = yps.tile([128, LCH], f32, name=f"yp{t}_{c}", tag="yp")
                        for k in range(K):
                            nc.tensor.matmul(
                                yp[:],
                                wpk[:, k * 128:(k + 1) * 128],
                                xp[:, c * LCH + k:c * LCH + k + LCH],
                                start=(k == 0),
                                stop=(k == K - 1),
                            )
                        # bias fused into the PSUM->SBUF copy, alternating
                        # DVE / ACT so neither engine is the bottleneck
                        if c % 2 == 0:
                            nc.vector.tensor_scalar_add(
                                yo[:, cc * LCH:(cc + 1) * LCH], yp[:], bias_sb[:]
                            )
                        else:
                            nc.scalar.activation(
                                yo[:, cc * LCH:(cc + 1) * LCH], yp[:],
                                AF.Identity, bias=bias_sb[:],
                            )
                    # ~0.5MB bf16 output blocks: both samples in one
                    # 128-partition DMA on the scalar HWDGE ring
                    lo, hi = ob * OB, (ob + 1) * OB
                    nc.scalar.dma_start(Y[2 * t:2 * t + 2, :, lo:hi], yo[:])

    nc.compile()
    return nc


def _host_prep(inputs):
    """Shared (replicated) tensors, in device layouts. Returns dict of np arrays."""
    f = np.float32
    gA_flat = (inputs["A_bn_g"] / np.sqrt(f(1.0) + f(BN_EPS))).astype(f)
    gB_flat = (inputs["B_bn_g"] / np.sqrt(f(1.0) + f(BN_EPS))).astype(f)
    cA_flat = (inputs["A_b1"] * gA_flat + inputs["A_bn_b"]).astype(f)
    cB_flat = (inputs["B_b1"] * gB_flat + inputs["B_bn_b"]).astype(f)

    # A layer-2: columns m = i*8+r  ->  m' = r*64+i (r-major)
    permA = (np.arange(R)[:, None] + np.arange(CIN)[None, :] * R).reshape(-1)  # m'[r,i] -> i*8+r
    Aw2T = np.ascontiguousarray(inputs["A_w2"].T[:, permA], dtype=f)
    b2A = np.ascontiguousarray(inputs["A_b2"][permA], dtype=f).reshape(1, CIN * R)

    # B layer-2: columns m = r*192 + cout*3 + k  ->  m' = r*192 + k*64 + cout
    m2 = (np.arange(COUT)[None, :] * K + np.arange(K)[:, None]).reshape(-1)  # m2'[k,c] -> c*3+k
    permB = (np.arange(R)[:, None] * KCO + m2[None, :]).reshape(-1)
    Bw2T = np.ascontiguousarray(inputs["B_w2"].T[:, permB], dtype=f)
    b2B = np.ascontiguousarray(inputs["B_b2"][permB], dtype=f).reshape(1, R * KCO)

    # base_w [cout, cin, k] -> tap-major block-diag pair layout:
    # base_pair[j*64 + i, k*128 + j*64 + c] = base_w[c, i, k]
    base_pair = np.zeros((128, 2 * KCO), dtype=f)
    for j in range(2):
        for k in range(K):
            base_pair[j * 64:(j + 1) * 64, k * 128 + j * 64:k * 128 + j * 64 + 64] = (
                inputs["base_w"][:, :, k].T.astype(f)
            )

    bias_out = np.concatenate([inputs["base_b"], inputs["base_b"]]).astype(f)

    # all per-partition vectors in one tensor -> one early DMA:
    # cols = gA0 gA1 cA0 cA1 gB0 gB1 cB0 cB1 bias_out
    vecs = np.stack([
        gA_flat[:128], gA_flat[128:], cA_flat[:128], cA_flat[128:],
        gB_flat[:128], gB_flat[128:], cB_flat[:128], cB_flat[128:],
        bias_out,
    ], axis=1).astype(f)

    return {
        "Aw1T": np.ascontiguousarray(inputs["A_w1"].T, dtype=f),
        "Bw1T": np.ascontiguousarray(inputs["B_w1"].T, dtype=f),
        "Aw2T": Aw2T,
        "Bw2T": Bw2T,
        "vecs": vecs,
        "b2A": b2A,
        "b2B": b2B,
        "base_pair": base_pair,
    }


def _in_maps(inputs):
    shared = _host_prep(inputs)
    f = np.float32
    maps = []
    for c in range(NCORES):
        lo, hi = c * SH, (c + 1) * SH
        m = dict(shared)
        m["X"] = np.ascontiguousarray(inputs["X"][lo:hi]).astype(BF16)
        m["aT"] = np.ascontiguousarray(inputs["a_embedding"][lo:hi].T, dtype=f)
        m["bT"] = np.ascontiguousarray(inputs["b_embedding"][lo:hi].T, dtype=f)
        maps.append(m)
    return maps


def run(inputs, trace=False):
    """Run the kernel; returns (Y_full, BassKernelResults)."""
    global _NC
    if _NC is None:
        _NC = _build_program()
    from concourse.bass_utils import run_bass_kernel_spmd

    res = run_bass_kernel_spmd(
        _NC, _in_maps(inputs), core_ids=list(range(NCORES)), trace=trace
    )
    Y = np.concatenate([r["Y"] for r in res.results], axis=0).astype(np.float32)
    return Y, res


def kernel(**inputs) -> np.ndarray:
    Y, _ = run(inputs, trace=False)
    return Y

